# revision 22
# baseline (speedup 1.0000x reference)
"""Distributed GQA attention block (dense_transformer) for 8 TRN2 NeuronCores.

Reference computation (all fp32):
    q = (x @ wq)  -> RoPE;  k = (x @ wk) -> RoPE;  v = x @ wv
    causal softmax(q k^T / sqrt(64)) @ v  (GQA: 32 q heads, 4 kv heads)
    out = attn_out @ wo
Sharding: core (b, g) for b in {0,1}, g in {0..3} handles batch b, q-heads
8g..8g+7, kv-head g (data-parallel over batch x tensor-parallel over GQA
groups).  Each core computes attn_outT for its heads ([512, 2048],
feature-major), AllGathers within its 4-core batch group, and applies a
512-column slice of wo.  Outputs are disjoint -> host concat only.

Layout/scheduling notes:
  - All inputs host-pre-swizzled and loaded as a handful of large flat
    contiguous DMAs (intro is HBM-bandwidth bound, not issue bound).
  - Attention-phase PSUM evacuations ride the Vector engine; the Scalar
    engine is reserved for the softmax exps (it is the phase bottleneck).
  - The wo projection accumulates head-pairs 0..2 into bf16 partials while
    the ph3 gather is in flight; only the last head-pair's 4 matmuls and
    one vector add land after it.
"""

import json

import numpy as np
import ml_dtypes

import concourse.bass as bass
import concourse.bass2jax as bass2jax
import concourse.mybir as mybir
import concourse.tile as tile
from concourse.tile import VectorClock, ScopedClock
from concourse.bass_utils import compile_bir_kernel, run_bass_kernel_spmd

_MAX_WAITS = 1  # this walrus build rejects instructions with more sem waits


def _split_excess_waits(bir_json, max_waits=_MAX_WAITS):
    """Hoist excess per-instruction sem waits onto injected same-engine NoOps.

    The TRN2 ISA encoding in this neuronxcc build allows at most `max_waits`
    sync-wait commands per instruction; Tile's sem assigner can emit more.
    A NoOp inserted immediately before the instruction on the same engine is
    semantically identical (the engine blocks at the same program point).
    """
    d = json.loads(bir_json)
    changed = False
    for fn in d.get("functions", []):
        for bb in fn.get("blocks", []):
            insts = bb.get("instructions", [])
            new = []
            for ins in insts:
                si = ins.get("sync_info")
                waits = (si or {}).get("on_wait") or []
                if len(waits) > max_waits:
                    changed = True
                    excess, keep = waits[:-max_waits], waits[-max_waits:]
                    for i in range(0, len(excess), max_waits):
                        new.append(
                            {
                                "debug": ins.get("debug", 0),
                                "engine": ins["engine"],
                                "ins": [],
                                "name": f"{ins['name']}-wsplit{i}",
                                "opcode": "NoOp",
                                "outs": [],
                                "sync_info": {
                                    "on_update": [],
                                    "on_wait": excess[i : i + max_waits],
                                },
                            }
                        )
                    si["on_wait"] = keep
                new.append(ins)
            bb["instructions"] = new
    if not changed:
        return bir_json
    return json.dumps(d).encode()


def _patched_compile_bir_kernel(bir_json, tmpdir, neff_name="file.neff"):
    return compile_bir_kernel(_split_excess_waits(bir_json), tmpdir, neff_name)


bass2jax.compile_bir_kernel = _patched_compile_bir_kernel

BF16 = ml_dtypes.bfloat16
F32 = mybir.dt.float32
BF = mybir.dt.bfloat16

DIM = 2048
T = 2048
HD = 64
N_CORES = 8
AF = mybir.ActivationFunctionType


class _TileContext(tile.TileContext):
    """TileContext whose final drain carries one sem wait per instruction.

    The walrus build in this image rejects a Drain carrying several sync
    waits ("Too many sync wait commands"), so emit individual single-wait
    NOPs on the sync engine first, then an unadorned drain + barriers.
    """

    def _drain_and_barrier(self, tick_clock, wait_clock):
        gc = tick_clock.global_clock
        vals = eval(repr(gc).replace("VectorClock(", "").rstrip(")"))
        for i, v in enumerate(vals):
            if v:
                single = [0] * len(vals)
                single[i] = v
                nop = self.nc.sync.nop(nofuse=True)
                wait_clock.add_sem_waits(
                    nop.ins, ScopedClock({None: VectorClock(single)})
                )
        self.nc.sync.drain()
        self.nc.all_engine_barrier()
        popped = self.nc._tile_sem_poison_stack.pop()
        assert popped is self._sem_poison
        self.nc.clear_and_free_semaphores(list(self.sems.allocated().values()))
        self.nc.all_engine_barrier()


def _build_nc():
    nc = bass.Bass("TRN2")

    # host-pre-swizzled inputs: one flat contiguous DMA each
    xt = nc.declare_dram_parameter("xt", [128, 4 * 16 * 512], BF, isOutput=False)
    wq = nc.declare_dram_parameter("wq", [128, 16 * 4 * 128], BF, isOutput=False)
    wkv = nc.declare_dram_parameter("wkv", [128, 16 * 128], BF, isOutput=False)
    wo = nc.declare_dram_parameter("wo", [128, 16 * 4 * 128], BF, isOutput=False)
    ctabA = nc.declare_dram_parameter(
        "ctabA", [128, 3 * 2048], BF, isOutput=False
    )  # coskv | sinkv | masks
    ctabB = nc.declare_dram_parameter(
        "ctabB", [128, 2 * 2048], BF, isOutput=False
    )  # cos2 | sin2
    rident = nc.declare_dram_parameter("rident", [128, 192], BF, isOutput=False)
    outt = nc.declare_dram_parameter("outt", [512, T], BF, isOutput=True)

    with _TileContext(nc) as tc:
        with (
            tc.tile_pool(name="consts", bufs=1) as consts,
            tc.tile_pool(name="big", bufs=1) as big,
            tc.tile_pool(name="wts", bufs=1) as wts,
            tc.tile_pool(name="acts", bufs=1) as acts,
            tc.tile_pool(name="work", bufs=4) as work,
            tc.tile_pool(name="exps", bufs=6) as exps,
            tc.tile_pool(name="outp", bufs=3) as outp,
            tc.tile_pool(name="psum", bufs=3, space="PSUM") as psum,
            tc.tile_pool(name="dram", bufs=1, space="DRAM") as dram,
        ):
            # ---- constants (rident first: it feeds the PE warm-up burst) ----
            rident_sb = consts.tile([128, 192], BF)
            nc.sync.dma_start(rident_sb[:], rident[:])
            r2t_sb = rident_sb[:, 0:128]

            # PE warm-up: back-to-back matmuls during the DMA intro lift the
            # HAM clock gate to 2.4 GHz before real compute starts; sized to
            # cover until the first xt chunk lands so the PE never re-chills
            pwarm = psum.tile([128, 512], F32, tag="mm", name="pwarm", bufs=2)
            for wi in range(130):
                nc.tensor.matmul(
                    pwarm[:, 0:128], lhsT=r2t_sb, rhs=r2t_sb,
                    start=True, stop=True,
                )

            # ---- activations / weights in (sync ring: wkv, xt; scalar ring:
            # tables, wq; wo streams later mid-attention) ----
            wkv_sb = wts.tile([128, 16 * 128], BF)
            nc.sync.dma_start(wkv_sb[:], wkv[:])
            xt_sb = big.tile([128, 4 * 16 * 512], BF, tag="big")
            nc.scalar.dma_start(xt_sb[:, 0:8192], xt[:, 0:8192])
            for tt in range(1, 4):
                nc.sync.dma_start(
                    xt_sb[:, tt * 8192 : tt * 8192 + 8192],
                    xt[:, tt * 8192 : tt * 8192 + 8192],
                )
            ctabA_sb = consts.tile([128, 3 * 2048], BF)
            nc.scalar.dma_start(ctabA_sb[:], ctabA[:])
            wq_sb = wts.tile([128, 16 * 4 * 128], BF)
            nc.scalar.dma_start(wq_sb[:], wq[:])
            ctabB_sb = consts.tile([128, 2 * 2048], BF)
            nc.scalar.dma_start(ctabB_sb[:], ctabB[:])
            wo_sb = wts.tile([128, 16 * 4 * 128], BF)

            def xts(tt, fc):
                return xt_sb[:, tt * 8192 + fc * 512 : tt * 8192 + fc * 512 + 512]

            # ---- kv projection + rope (k rows 0..63, v rows 64..127) ----
            # prep(tt) produces everything the qb=tt attention units of ph0
            # need: roped k (duplicated into both PE row halves), v1 chunks
            # 4tt..4tt+3, interleaved into the ph0 stream right behind the
            # per-tt xt DMA so the softmax pipeline starts early
            kvrope_sb = acts.tile([128, T], BF)
            kdup_sb = acts.tile([128, T], BF)
            v1_sb = acts.tile([128, 16, 65], BF)
            nc.vector.memset(v1_sb[:, :, 64:65], 1.0)

            def emit_prep(tt):
                t0 = tt * 512
                ps = psum.tile([128, 512], F32, tag="mm", bufs=2)
                for fc in range(16):
                    nc.tensor.matmul(
                        ps[:],
                        lhsT=wkv_sb[:, fc * 128 : fc * 128 + 128],
                        rhs=xts(tt, fc),
                        start=(fc == 0),
                        stop=(fc == 15),
                    )
                kv_sb = work.tile([128, 512], BF, tag="evac")
                nc.vector.tensor_copy(kv_sb[:], ps[:])
                psu = psum.tile([128, 512], F32, tag="mm", name="psu", bufs=2)
                nc.tensor.matmul(
                    psu[:], lhsT=r2t_sb, rhs=kv_sb[:], start=True, stop=True
                )
                t1 = work.tile([128, 512], BF, tag="t1")
                nc.vector.tensor_mul(t1[:], kv_sb[:], ctabA_sb[:, t0 : t0 + 512])
                t2 = work.tile([128, 512], BF, tag="t2")
                nc.vector.tensor_mul(
                    t2[:], psu[:], ctabA_sb[:, 2048 + t0 : 2048 + t0 + 512]
                )
                nc.vector.tensor_add(kvrope_sb[:, t0 : t0 + 512], t1[:], t2[:])
                nc.scalar.dma_start(
                    kdup_sb[0:64, t0 : t0 + 512], kvrope_sb[0:64, t0 : t0 + 512]
                )
                nc.scalar.dma_start(
                    kdup_sb[64:128, t0 : t0 + 512], kvrope_sb[0:64, t0 : t0 + 512]
                )
                for kt in range(4 * tt, 4 * tt + 4):
                    pst = psum.tile([128, 64], BF, tag="mm", bufs=2, name="pst")
                    nc.tensor.transpose(
                        pst[:],
                        kvrope_sb[64:128, kt * 128 : kt * 128 + 128],
                        rident_sb[64:128, 128:192],
                    )
                    nc.vector.tensor_copy(v1_sb[:, kt, 0:64], pst[:])

            # ---- q projection chunks interleaved with attention head pairs ----
            qrope_sb = acts.tile([128, 4, T], BF)
            ao_q = [dram.tile([128, T], BF, name=f"aoq{i}") for i in range(4)]
            aof_q = [dram.tile([512, T], BF, name=f"aofq{i}") for i in range(4)]
            scale = 1.0 / np.sqrt(HD)
            aof_sb = big.tile([128, 16 * T], BF, tag="big")

            def emit_norm(u):
                # evacuate unnormalized av + denominators (one copy per
                # half), releasing the PSUM accumulators; the rest runs off
                # the critical path (DRAM-bounce broadcast + compact
                # reciprocal) with no PE/PSUM involvement
                uph, uqb, upav = u
                uQ0 = uqb * 512
                avu = []
                for par in range(2):
                    avu_sb = work.tile([65, 512], BF, tag="avu", name=f"avu{par}")
                    nc.vector.tensor_copy(avu_sb[:], upav[par][:])
                    avu.append(avu_sb)
                dden = dram.tile([2, 512], BF, tag="dden", bufs=4, name="dden")
                for par in range(2):
                    nc.sync.dma_start(dden[par : par + 1, :], avu[par][64:65, :])
                rden_sb = work.tile([8, 128], BF, tag="rden")
                nc.sync.dma_start(
                    rden_sb[:],
                    bass.AP(tensor=dden.tensor, offset=dden.offset,
                            ap=[[128, 8], [1, 128]]),
                )
                with nc.allow_low_precision(
                    reason="bf16 softmax denominators are within tolerance"
                ):
                    nc.vector.reciprocal(rden_sb[:], rden_sb[:])
                rdden = dram.tile([2, 512], BF, tag="rdden", bufs=4, name="rdden")
                nc.sync.dma_start(
                    bass.AP(tensor=rdden.tensor, offset=rdden.offset,
                            ap=[[128, 8], [1, 128]]),
                    rden_sb[:],
                )
                for par in range(2):
                    b_sb = work.tile([64, 512], BF, tag="bcast", name=f"b{par}")
                    nc.sync.dma_start(
                        b_sb[:],
                        bass.AP(
                            tensor=rdden.tensor,
                            offset=rdden[par : par + 1, :].offset,
                            ap=[[0, 64], [1, 512]],
                        ),
                    )
                    av_sb = work.tile([64, 512], BF, tag="av", name=f"av{par}")
                    nc.vector.tensor_mul(av_sb[:], avu[par][0:64, :], b_sb[:])
                    nc.sync.dma_start(
                        ao_q[uph][64 * par : 64 * par + 64, uQ0 : uQ0 + 512],
                        av_sb[:],
                    )

            def emit_qproj(ph, tt):
                t0 = tt * 512
                ps = psum.tile([128, 512], F32, tag="mm", name="psq", bufs=2)
                for fc in range(16):
                    nc.tensor.matmul(
                        ps[:],
                        lhsT=wq_sb[
                            :, fc * 512 + ph * 128 : fc * 512 + ph * 128 + 128
                        ],
                        rhs=xts(tt, fc),
                        start=(fc == 0),
                        stop=(fc == 15),
                    )
                q_sb = work.tile([128, 512], BF, tag="evac")
                nc.vector.tensor_copy(q_sb[:], ps[:])
                psu = psum.tile([128, 512], F32, tag="mm", name="psu2", bufs=2)
                nc.tensor.matmul(
                    psu[:], lhsT=r2t_sb, rhs=q_sb[:], start=True, stop=True
                )
                t1 = work.tile([128, 512], BF, tag="t1")
                nc.vector.tensor_mul(t1[:], q_sb[:], ctabB_sb[:, t0 : t0 + 512])
                t2 = work.tile([128, 512], BF, tag="t2")
                nc.vector.tensor_mul(
                    t2[:], psu[:], ctabB_sb[:, 2048 + t0 : 2048 + t0 + 512]
                )
                nc.vector.tensor_add(qrope_sb[:, ph, t0 : t0 + 512], t1[:], t2[:])
                if ph == 1 and tt == 0:
                    # stream wo weights mid-attention on the scalar HWDGE
                    # queue; no waits, so ACT is not blocked
                    nc.scalar.dma_start(wo_sb[:], wo[:])
                if ph == 3 and tt == 3:
                    # xt is dead after this block: reload the first two
                    # gathered head pairs into its SBUF slot (scalar ring;
                    # pairs 2/3 reload as their gathers complete)
                    for i in range(2):
                        nc.gpsimd.dma_start(
                            aof_sb[:, i * 8192 : i * 8192 + 8192].rearrange(
                                "p (c t) -> p c t", t=2048
                            ),
                            aof_q[i][:, :].rearrange("(c p) t -> p c t", p=128),
                        )

            # attention unit (ph, qb, pr): scores + exps emitted immediately,
            # the AV matmuls one unit later (so a stalled AV never head-of-
            # line-blocks the next unit's scores in the PE queue)
            avstate = {"pav": None}

            ucount = {"n": 0}

            def emit_scores_exp(ph, qb, pr):
                Q0 = qb * 512
                ucount["n"] += 1
                pars = (0, 1)
                kt0, kt1 = 2 * pr, 2 * pr + 1
                # causal-active widths (tiles above the diagonal shrink)
                j0, j1 = kt0 - 4 * qb, kt1 - 4 * qb
                w0 = 512 if j0 < 0 else 512 - 128 * j0
                w1 = 512 if j1 < 0 else 512 - 128 * j1
                diag = j0 >= 0
                # scores for both head halves interleaved so adjacent
                # matmuls target different PE row groups (concurrent)
                pss = [
                    psum.tile([128, 1024], F32, tag="pss", name=f"pss{i}", bufs=2)
                    for i in range(2)
                ]
                for kt, w, off in ((kt0, w0, 0), (kt1, w1, w0)):
                    for par in pars:
                        lo, hi = (0, 64) if par == 0 else (64, 128)
                        nc.tensor.matmul(
                            pss[par][:, off : off + w],
                            lhsT=kdup_sb[lo:hi, kt * 128 : kt * 128 + 128],
                            rhs=qrope_sb[lo:hi, ph, Q0 + 512 - w : Q0 + 512],
                            start=True,
                            stop=True,
                        )
                e_pair = [None, None]
                for par in pars:
                    e_sb = exps.tile([128, 1024], BF, tag="e", name=f"e{par}")
                    nc.scalar.activation(
                        e_sb[:, 0 : w0 + w1], pss[par][:, 0 : w0 + w1],
                        AF.Exp, scale=scale,
                    )
                    if diag:
                        # causal mask (c >= r) is all-ones beyond col 127:
                        # only the leading 128 cols of each tile need it
                        m0, m1 = min(w0, 128), min(w1, 128)
                        nc.vector.tensor_mul(
                            e_sb[:, 0:m0], e_sb[:, 0:m0],
                            ctabA_sb[:, 4096 : 4096 + m0],
                        )
                        nc.vector.tensor_mul(
                            e_sb[:, w0 : w0 + m1], e_sb[:, w0 : w0 + m1],
                            ctabA_sb[:, 4096 : 4096 + m1],
                        )
                    e_pair[par] = e_sb
                return (ph, qb, pr, e_pair, ((kt0, w0, 0), (kt1, w1, w0)), pars)

            def flush_av(u):
                ph, qb, pr, e_pair, kts, pars = u
                nkt = 4 * qb + 4
                if pr == 0:
                    avstate["pav"] = [
                        psum.tile([65, 512], F32, tag="pav", name=f"pav{i}", bufs=2)
                        for i in range(2)
                    ]
                pav = avstate["pav"]
                for kt, w, off in kts:
                    for par in pars:
                        nc.tensor.matmul(
                            pav[par][:, 512 - w : 512],
                            lhsT=v1_sb[:, kt, :],
                            rhs=e_pair[par][:, off : off + w],
                            start=(kt == 0),
                            stop=(kt == nkt - 1),
                        )
                if pr == nkt // 2 - 1:
                    emit_norm((ph, qb, pav))
                    if qb == 3:
                        # gather this head pair while later pairs compute
                        nc.gpsimd.collective_compute(
                            "AllGather", mybir.AluOpType.bypass,
                            ins=[ao_q[ph].opt()], outs=[aof_q[ph].opt()],
                            replica_groups=[[0, 1, 2, 3], [4, 5, 6, 7]],
                        )
                        if ph == 3:
                            for i in (2, 3):
                                nc.gpsimd.dma_start(
                                    aof_sb[:, i * 8192 : i * 8192 + 8192].rearrange(
                                        "p (c t) -> p c t", t=2048
                                    ),
                                    aof_q[i][:, :].rearrange(
                                        "(c p) t -> p c t", p=128
                                    ),
                                )

            # build the emission stream: attention units in order, with the
            # q projections interleaved (qp(ph,0/1) late in ph-1's stream,
            # qp(ph,2/3) after ph's qb0/qb1) so the PE never sees a long
            # exp-free projection block while ACT idles
            stream = []
            for ph in range(4):
                att = []
                for qb in range(4):
                    if ph == 0:
                        att.append(("prep", qb))
                        att.append(("qp", 0, qb))
                    att += [("att", ph, qb, pr) for pr in range(2 * qb + 2)]
                if ph == 0:
                    att.insert(len(att) - 10, ("qp", 1, 0))
                    att.insert(len(att) - 5, ("qp", 1, 1))
                else:
                    att.insert(2, ("qp", ph, 2))
                    att.insert(7, ("qp", ph, 3))
                    if ph < 3:
                        att.insert(12, ("qp", ph + 1, 0))
                        att.insert(17, ("qp", ph + 1, 1))
                stream += att
            staged = None
            for it in stream:
                if it[0] == "prep":
                    emit_prep(it[1])
                elif it[0] == "qp":
                    emit_qproj(it[1], it[2])
                else:
                    u = emit_scores_exp(it[1], it[2], it[3])
                    if staged is not None:
                        flush_av(staged)
                    staged = u
            flush_av(staged)

            # ---- output projection (512-column slice of wo) ----
            # aof chunk 4*i+c covers rank c, head pair i -> wo feature-chunk
            # 4*c+i.  Partials accumulate incrementally as each gather's
            # reload lands (pairs 0+1, then +2); only pair 3's 4 matmuls and
            # one vector add trail the final gather.
            def wos(fc, cc):
                return wo_sb[:, fc * 512 + cc * 128 : fc * 512 + cc * 128 + 128]

            def aofs(ch, t0):
                return aof_sb[:, ch * 2048 + t0 : ch * 2048 + t0 + 512]

            wo_part = acts.tile([128, 16, 512], BF)
            for cc in range(4):
                for tt in range(4):
                    ps = psum.tile([128, 512], F32, tag="mm", bufs=2)
                    n = 0
                    for i in range(2):
                        for c in range(4):
                            nc.tensor.matmul(
                                ps[:],
                                lhsT=wos(4 * c + i, cc),
                                rhs=aofs(4 * i + c, tt * 512),
                                start=(n == 0),
                                stop=(n == 7),
                            )
                            n += 1
                    nc.vector.tensor_copy(wo_part[:, 4 * cc + tt, :], ps[:])
            for cc in range(4):
                for tt in range(4):
                    ps = psum.tile([128, 512], F32, tag="mm", bufs=2)
                    for c in range(4):
                        nc.tensor.matmul(
                            ps[:],
                            lhsT=wos(4 * c + 2, cc),
                            rhs=aofs(8 + c, tt * 512),
                            start=(c == 0),
                            stop=(c == 3),
                        )
                    nc.vector.tensor_add(
                        wo_part[:, 4 * cc + tt, :], ps[:], wo_part[:, 4 * cc + tt, :]
                    )
            for tt in range(4):
                for cc in range(4):
                    ts = slice(tt * 512, tt * 512 + 512)
                    ps = psum.tile([128, 512], F32, tag="mm", bufs=2)
                    for c in range(4):
                        nc.tensor.matmul(
                            ps[:],
                            lhsT=wos(4 * c + 3, cc),
                            rhs=aofs(12 + c, tt * 512),
                            start=(c == 0),
                            stop=(c == 3),
                        )
                    o_sb = outp.tile([128, 512], BF, tag="o")
                    nc.vector.tensor_add(o_sb[:], ps[:], wo_part[:, 4 * cc + tt, :])
                    nc.sync.dma_start(outt[cc * 128 : cc * 128 + 128, ts], o_sb[:])

    return nc


def _host_tables():
    inv_freq = 1.0 / (10000.0 ** (np.arange(0, HD, 2, dtype=np.float32) / HD))
    t = np.arange(T, dtype=np.float32)
    freqs = np.einsum("i,j->ij", t, inv_freq)
    emb = np.concatenate([freqs, freqs], axis=-1)  # [T, 64]
    cosT = np.cos(emb).T.astype(np.float32)  # [64, T]
    sinT = np.sin(emb).T.astype(np.float32)

    cos2 = np.ascontiguousarray(np.vstack([cosT, cosT]))
    sin2 = np.ascontiguousarray(np.vstack([sinT, sinT]))
    coskv = np.ascontiguousarray(np.vstack([cosT, np.ones_like(cosT)]))
    sinkv = np.ascontiguousarray(np.vstack([sinT, np.zeros_like(sinT)]))

    masks = np.zeros((128, T), dtype=np.float32)
    r_idx = np.arange(128)[:, None]
    c_idx = np.arange(512)[None, :]
    for j in range(4):
        masks[:, j * 512 : j * 512 + 512] = (c_idx >= 128 * j + r_idx)

    ctabA = np.concatenate([coskv, sinkv, masks], axis=1).astype(BF16)
    ctabB = np.concatenate([cos2, sin2], axis=1).astype(BF16)

    R = np.zeros((HD, HD), dtype=np.float32)
    for d in range(32):
        R[d, d + 32] = -1.0
        R[d + 32, d] = 1.0
    r2 = np.block([[R, np.zeros_like(R)], [np.zeros_like(R), R]])
    r2t = np.ascontiguousarray(r2.T)  # lhsT: matmul computes R2 @ rhs

    ident2 = np.vstack([np.eye(HD), np.eye(HD)])  # [128, 64]
    rident = np.concatenate([r2t, ident2], axis=1).astype(BF16)  # [128, 192]

    return dict(ctabA=ctabA, ctabB=ctabB, rident=rident)


def _swz(w):
    """[2048, n] -> [128, 16*n] chunk-major swizzle (chunk fc at col fc*n)."""
    n = w.shape[1]
    return np.ascontiguousarray(
        w.reshape(16, 128, n).transpose(1, 0, 2).reshape(128, 16 * n)
    )


def prepare_in_maps(x, wq, wk, wv, wo):
    tables = _get_nc()[1]
    x = np.asarray(x, dtype=np.float32)
    wq = np.asarray(wq, dtype=np.float32)
    wk = np.asarray(wk, dtype=np.float32)
    wv = np.asarray(wv, dtype=np.float32)
    wo = np.asarray(wo, dtype=np.float32)

    # xt layout [p][tt][fc][512]
    xts = []
    for b in range(2):
        xts.append(
            np.ascontiguousarray(
                x[b]
                .reshape(4, 512, 16, 128)
                .transpose(3, 0, 2, 1)
                .reshape(128, 4 * 16 * 512)
            ).astype(BF16)
        )

    in_maps = []
    for core in range(N_CORES):
        b, g = core // 4, core % 4
        m = dict(tables)
        m["xt"] = xts[b]
        m["wq"] = _swz(wq[:, 512 * g : 512 * g + 512]).astype(BF16)
        m["wo"] = _swz(wo[:, 512 * g : 512 * g + 512]).astype(BF16)
        m["wkv"] = _swz(
            np.concatenate(
                [wk[:, 64 * g : 64 * g + 64], wv[:, 64 * g : 64 * g + 64]],
                axis=1,
            )
        ).astype(BF16)
        in_maps.append(m)
    return in_maps


def gather_output(res):
    out = np.empty((2, T, DIM), dtype=np.float32)
    for core in range(N_CORES):
        b, g = core // 4, core % 4
        out[b][:, 512 * g : 512 * g + 512] = (
            res.results[core]["outt"].astype(np.float32).T
        )
    return out


_STATE = {}


def _get_nc():
    if "nc" not in _STATE:
        _STATE["tables"] = _host_tables()
        _STATE["nc"] = _build_nc()
    return _STATE["nc"], _STATE["tables"]


def kernel(x, wq, wk, wv, wo):
    nc, _ = _get_nc()
    in_maps = prepare_in_maps(x, wq, wk, wv, wo)
    res = run_bass_kernel_spmd(
        nc, in_maps, core_ids=list(range(N_CORES)), trace=False
    )
    return gather_output(res)


# revision 23
# speedup vs baseline: 1.0507x; 1.0507x over previous
"""Distributed GQA attention block (dense_transformer) for 8 TRN2 NeuronCores.

Reference computation (all fp32):
    q = (x @ wq)  -> RoPE;  k = (x @ wk) -> RoPE;  v = x @ wv
    causal softmax(q k^T / sqrt(64)) @ v  (GQA: 32 q heads, 4 kv heads)
    out = attn_out @ wo
Sharding: core (b, g) for b in {0,1}, g in {0..3} handles batch b, q-heads
8g..8g+7, kv-head g (data-parallel over batch x tensor-parallel over GQA
groups).  Each core computes attn_outT for its heads ([512, 2048],
feature-major), AllGathers within its 4-core batch group, and applies a
512-column slice of wo.  Outputs are disjoint -> host concat only.

Layout/scheduling notes:
  - All inputs host-pre-swizzled and loaded as a handful of large flat
    contiguous DMAs (intro is HBM-bandwidth bound, not issue bound).
  - Attention-phase PSUM evacuations ride the Vector engine; the Scalar
    engine is reserved for the softmax exps (it is the phase bottleneck).
  - The wo projection accumulates head-pairs 0..2 into bf16 partials while
    the ph3 gather is in flight; only the last head-pair's 4 matmuls and
    one vector add land after it.
"""

import json

import numpy as np
import ml_dtypes

import concourse.bass as bass
import concourse.bass2jax as bass2jax
import concourse.mybir as mybir
import concourse.tile as tile
from concourse.tile import VectorClock, ScopedClock
from concourse.bass_utils import compile_bir_kernel, run_bass_kernel_spmd

_MAX_WAITS = 1  # this walrus build rejects instructions with more sem waits


def _split_excess_waits(bir_json, max_waits=_MAX_WAITS):
    """Hoist excess per-instruction sem waits onto injected same-engine NoOps.

    The TRN2 ISA encoding in this neuronxcc build allows at most `max_waits`
    sync-wait commands per instruction; Tile's sem assigner can emit more.
    A NoOp inserted immediately before the instruction on the same engine is
    semantically identical (the engine blocks at the same program point).
    """
    d = json.loads(bir_json)
    changed = False
    for fn in d.get("functions", []):
        for bb in fn.get("blocks", []):
            insts = bb.get("instructions", [])
            new = []
            for ins in insts:
                si = ins.get("sync_info")
                waits = (si or {}).get("on_wait") or []
                if len(waits) > max_waits:
                    changed = True
                    excess, keep = waits[:-max_waits], waits[-max_waits:]
                    for i in range(0, len(excess), max_waits):
                        new.append(
                            {
                                "debug": ins.get("debug", 0),
                                "engine": ins["engine"],
                                "ins": [],
                                "name": f"{ins['name']}-wsplit{i}",
                                "opcode": "NoOp",
                                "outs": [],
                                "sync_info": {
                                    "on_update": [],
                                    "on_wait": excess[i : i + max_waits],
                                },
                            }
                        )
                    si["on_wait"] = keep
                new.append(ins)
            bb["instructions"] = new
    if not changed:
        return bir_json
    return json.dumps(d).encode()


def _patched_compile_bir_kernel(bir_json, tmpdir, neff_name="file.neff"):
    return compile_bir_kernel(_split_excess_waits(bir_json), tmpdir, neff_name)


bass2jax.compile_bir_kernel = _patched_compile_bir_kernel

BF16 = ml_dtypes.bfloat16
F32 = mybir.dt.float32
BF = mybir.dt.bfloat16

DIM = 2048
T = 2048
HD = 64
N_CORES = 8
AF = mybir.ActivationFunctionType


class _TileContext(tile.TileContext):
    """TileContext whose final drain carries one sem wait per instruction.

    The walrus build in this image rejects a Drain carrying several sync
    waits ("Too many sync wait commands"), so emit individual single-wait
    NOPs on the sync engine first, then an unadorned drain + barriers.
    """

    def _drain_and_barrier(self, tick_clock, wait_clock):
        gc = tick_clock.global_clock
        vals = eval(repr(gc).replace("VectorClock(", "").rstrip(")"))
        for i, v in enumerate(vals):
            if v:
                single = [0] * len(vals)
                single[i] = v
                nop = self.nc.sync.nop(nofuse=True)
                wait_clock.add_sem_waits(
                    nop.ins, ScopedClock({None: VectorClock(single)})
                )
        self.nc.sync.drain()
        self.nc.all_engine_barrier()
        popped = self.nc._tile_sem_poison_stack.pop()
        assert popped is self._sem_poison
        self.nc.clear_and_free_semaphores(list(self.sems.allocated().values()))
        self.nc.all_engine_barrier()


def _build_nc():
    nc = bass.Bass("TRN2")

    # host-pre-swizzled inputs: one flat contiguous DMA each
    xt = nc.declare_dram_parameter("xt", [128, 4 * 16 * 512], BF, isOutput=False)
    wq = nc.declare_dram_parameter("wq", [128, 16 * 4 * 128], BF, isOutput=False)
    wkv = nc.declare_dram_parameter("wkv", [128, 16 * 128], BF, isOutput=False)
    wo = nc.declare_dram_parameter("wo", [128, 16 * 4 * 128], BF, isOutput=False)
    ctabA = nc.declare_dram_parameter(
        "ctabA", [128, 3 * 2048], BF, isOutput=False
    )  # coskv | sinkv | masks
    ctabB = nc.declare_dram_parameter(
        "ctabB", [128, 2 * 2048], BF, isOutput=False
    )  # cos2 | sin2
    rident = nc.declare_dram_parameter("rident", [128, 192], BF, isOutput=False)
    outt = nc.declare_dram_parameter("outt", [512, T], BF, isOutput=True)

    with _TileContext(nc) as tc:
        with (
            tc.tile_pool(name="consts", bufs=1) as consts,
            tc.tile_pool(name="big", bufs=1) as big,
            tc.tile_pool(name="wts", bufs=1) as wts,
            tc.tile_pool(name="acts", bufs=1) as acts,
            tc.tile_pool(name="work", bufs=4) as work,
            tc.tile_pool(name="exps", bufs=6) as exps,
            tc.tile_pool(name="outp", bufs=3) as outp,
            tc.tile_pool(name="psum", bufs=3, space="PSUM") as psum,
            tc.tile_pool(name="dram", bufs=1, space="DRAM") as dram,
        ):
            # ---- constants (rident first: it feeds the PE warm-up burst) ----
            rident_sb = consts.tile([128, 192], BF)
            nc.sync.dma_start(rident_sb[:], rident[:])
            r2t_sb = rident_sb[:, 0:128]

            # PE warm-up: back-to-back matmuls during the DMA intro lift the
            # HAM clock gate to 2.4 GHz before real compute starts; sized to
            # cover until the first xt chunk lands so the PE never re-chills
            pwarm = psum.tile([128, 512], F32, tag="mm", name="pwarm", bufs=2)
            for wi in range(130):
                nc.tensor.matmul(
                    pwarm[:, 0:128], lhsT=r2t_sb, rhs=r2t_sb,
                    start=True, stop=True,
                )

            # ---- activations / weights in (sync ring: wkv, xt; scalar ring:
            # tables, wq; wo streams later mid-attention) ----
            wkv_sb = wts.tile([128, 16 * 128], BF)
            nc.sync.dma_start(wkv_sb[:], wkv[:])
            xt_sb = big.tile([128, 4 * 16 * 512], BF, tag="big")
            nc.scalar.dma_start(xt_sb[:, 0:8192], xt[:, 0:8192])
            for tt in range(1, 4):
                nc.sync.dma_start(
                    xt_sb[:, tt * 8192 : tt * 8192 + 8192],
                    xt[:, tt * 8192 : tt * 8192 + 8192],
                )
            ctabA_sb = consts.tile([128, 3 * 2048], BF)
            nc.scalar.dma_start(ctabA_sb[:], ctabA[:])
            wq_sb = wts.tile([128, 16 * 4 * 128], BF)
            nc.scalar.dma_start(wq_sb[:], wq[:])
            ctabB_sb = consts.tile([128, 2 * 2048], BF)
            nc.scalar.dma_start(ctabB_sb[:], ctabB[:])
            wo_sb = wts.tile([128, 16 * 4 * 128], BF)

            def xts(tt, fc):
                return xt_sb[:, tt * 8192 + fc * 512 : tt * 8192 + fc * 512 + 512]

            # ---- kv projection + rope (k rows 0..63, v rows 64..127) ----
            # prep(tt) produces everything the qb=tt attention units of ph0
            # need: roped k (duplicated into both PE row halves), v1 chunks
            # 4tt..4tt+3, interleaved into the ph0 stream right behind the
            # per-tt xt DMA so the softmax pipeline starts early
            kvrope_sb = acts.tile([128, T], BF)
            kdup_sb = acts.tile([128, T], BF)
            v1_sb = acts.tile([128, 16, 65], BF)
            nc.vector.memset(v1_sb[:, :, 64:65], 1.0)

            def emit_prep(tt):
                t0 = tt * 512
                ps = psum.tile([128, 512], F32, tag="mm", bufs=2)
                for fc in range(16):
                    nc.tensor.matmul(
                        ps[:],
                        lhsT=wkv_sb[:, fc * 128 : fc * 128 + 128],
                        rhs=xts(tt, fc),
                        start=(fc == 0),
                        stop=(fc == 15),
                    )
                kv_sb = work.tile([128, 512], BF, tag="evac")
                nc.vector.tensor_copy(kv_sb[:], ps[:])
                psu = psum.tile([128, 512], F32, tag="mm", name="psu", bufs=2)
                nc.tensor.matmul(
                    psu[:], lhsT=r2t_sb, rhs=kv_sb[:], start=True, stop=True
                )
                t1 = work.tile([128, 512], BF, tag="t1")
                nc.vector.tensor_mul(t1[:], kv_sb[:], ctabA_sb[:, t0 : t0 + 512])
                t2 = work.tile([128, 512], BF, tag="t2")
                nc.vector.tensor_mul(
                    t2[:], psu[:], ctabA_sb[:, 2048 + t0 : 2048 + t0 + 512]
                )
                nc.vector.tensor_add(kvrope_sb[:, t0 : t0 + 512], t1[:], t2[:])
                nc.scalar.dma_start(
                    kdup_sb[0:64, t0 : t0 + 512], kvrope_sb[0:64, t0 : t0 + 512]
                )
                nc.scalar.dma_start(
                    kdup_sb[64:128, t0 : t0 + 512], kvrope_sb[0:64, t0 : t0 + 512]
                )
                for kt in range(4 * tt, 4 * tt + 4):
                    pst = psum.tile([128, 64], BF, tag="mm", bufs=2, name="pst")
                    nc.tensor.transpose(
                        pst[:],
                        kvrope_sb[64:128, kt * 128 : kt * 128 + 128],
                        rident_sb[64:128, 128:192],
                    )
                    nc.vector.tensor_copy(v1_sb[:, kt, 0:64], pst[:])

            # ---- q projection chunks interleaved with attention head pairs ----
            qrope_sb = acts.tile([128, 4, T], BF)
            ao_q = [dram.tile([128, T], BF, name=f"aoq{i}") for i in range(4)]
            aof_q = [dram.tile([512, T], BF, name=f"aofq{i}") for i in range(4)]
            scale = 1.0 / np.sqrt(HD)
            aof_sb = big.tile([128, 16 * T], BF, tag="big")

            def emit_norm(u):
                # evacuate unnormalized av + denominators (one copy per
                # half), releasing the PSUM accumulators; the rest runs off
                # the critical path (DRAM-bounce broadcast + compact
                # reciprocal) with no PE/PSUM involvement
                uph, uqb, upav = u
                uQ0 = uqb * 512
                avu = []
                for par in range(2):
                    avu_sb = work.tile([65, 512], BF, tag="avu", name=f"avu{par}")
                    nc.vector.tensor_copy(avu_sb[:], upav[par][:])
                    avu.append(avu_sb)
                dden = dram.tile([2, 512], BF, tag="dden", bufs=4, name="dden")
                for par in range(2):
                    nc.sync.dma_start(dden[par : par + 1, :], avu[par][64:65, :])
                rden_sb = work.tile([8, 128], BF, tag="rden")
                nc.sync.dma_start(
                    rden_sb[:],
                    bass.AP(tensor=dden.tensor, offset=dden.offset,
                            ap=[[128, 8], [1, 128]]),
                )
                with nc.allow_low_precision(
                    reason="bf16 softmax denominators are within tolerance"
                ):
                    nc.vector.reciprocal(rden_sb[:], rden_sb[:])
                rdden = dram.tile([2, 512], BF, tag="rdden", bufs=4, name="rdden")
                nc.sync.dma_start(
                    bass.AP(tensor=rdden.tensor, offset=rdden.offset,
                            ap=[[128, 8], [1, 128]]),
                    rden_sb[:],
                )
                for par in range(2):
                    b_sb = work.tile([64, 512], BF, tag="bcast", name=f"b{par}")
                    nc.sync.dma_start(
                        b_sb[:],
                        bass.AP(
                            tensor=rdden.tensor,
                            offset=rdden[par : par + 1, :].offset,
                            ap=[[0, 64], [1, 512]],
                        ),
                    )
                    av_sb = work.tile([64, 512], BF, tag="av", name=f"av{par}")
                    nc.vector.tensor_mul(av_sb[:], avu[par][0:64, :], b_sb[:])
                    nc.sync.dma_start(
                        ao_q[uph][64 * par : 64 * par + 64, uQ0 : uQ0 + 512],
                        av_sb[:],
                    )

            def emit_qproj(ph, tt):
                t0 = tt * 512
                ps = psum.tile([128, 512], F32, tag="mm", name="psq", bufs=2)
                for fc in range(16):
                    nc.tensor.matmul(
                        ps[:],
                        lhsT=wq_sb[
                            :, fc * 512 + ph * 128 : fc * 512 + ph * 128 + 128
                        ],
                        rhs=xts(tt, fc),
                        start=(fc == 0),
                        stop=(fc == 15),
                    )
                q_sb = work.tile([128, 512], BF, tag="evac")
                nc.vector.tensor_copy(q_sb[:], ps[:])
                psu = psum.tile([128, 512], F32, tag="mm", name="psu2", bufs=2)
                nc.tensor.matmul(
                    psu[:], lhsT=r2t_sb, rhs=q_sb[:], start=True, stop=True
                )
                t1 = work.tile([128, 512], BF, tag="t1")
                nc.vector.tensor_mul(t1[:], q_sb[:], ctabB_sb[:, t0 : t0 + 512])
                t2 = work.tile([128, 512], BF, tag="t2")
                nc.vector.tensor_mul(
                    t2[:], psu[:], ctabB_sb[:, 2048 + t0 : 2048 + t0 + 512]
                )
                nc.vector.tensor_add(qrope_sb[:, ph, t0 : t0 + 512], t1[:], t2[:])
                if ph == 1 and tt == 0:
                    # stream wo weights mid-attention on the scalar HWDGE
                    # queue; no waits, so ACT is not blocked
                    nc.scalar.dma_start(wo_sb[:], wo[:])
                if ph == 3 and tt == 3:
                    # xt is dead after this block: reload the first two
                    # gathered head pairs into its SBUF slot (scalar ring;
                    # pairs 2/3 reload as their gathers complete)
                    for i in range(2):
                        nc.gpsimd.dma_start(
                            aof_sb[:, i * 8192 : i * 8192 + 8192].rearrange(
                                "p (c t) -> p c t", t=2048
                            ),
                            aof_q[i][:, :].rearrange("(c p) t -> p c t", p=128),
                        )

            # attention unit (ph, qb, pr): scores + exps emitted immediately,
            # the AV matmuls one unit later (so a stalled AV never head-of-
            # line-blocks the next unit's scores in the PE queue)
            avstate = {"pav": None}

            ucount = {"n": 0}

            def emit_scores_exp(ph, qb, pr):
                Q0 = qb * 512
                ucount["n"] += 1
                pars = (0, 1) if ucount["n"] % 2 else (1, 0)
                kt0, kt1 = 2 * pr, 2 * pr + 1
                # causal-active widths (tiles above the diagonal shrink)
                j0, j1 = kt0 - 4 * qb, kt1 - 4 * qb
                w0 = 512 if j0 < 0 else 512 - 128 * j0
                w1 = 512 if j1 < 0 else 512 - 128 * j1
                diag = j0 >= 0
                # scores for both head halves interleaved so adjacent
                # matmuls target different PE row groups (concurrent)
                pss = [None, None]
                for par in pars:
                    pss[par] = psum.tile(
                        [128, 1024], F32, tag="pss", name=f"pss{par}", bufs=2
                    )
                for kt, w, off in ((kt0, w0, 0), (kt1, w1, w0)):
                    for par in pars:
                        lo, hi = (0, 64) if par == 0 else (64, 128)
                        nc.tensor.matmul(
                            pss[par][:, off : off + w],
                            lhsT=kdup_sb[lo:hi, kt * 128 : kt * 128 + 128],
                            rhs=qrope_sb[lo:hi, ph, Q0 + 512 - w : Q0 + 512],
                            start=True,
                            stop=True,
                        )
                e_pair = [None, None]
                for par in pars:
                    e_sb = exps.tile([128, 1024], BF, tag="e", name=f"e{par}")
                    nc.scalar.activation(
                        e_sb[:, 0 : w0 + w1], pss[par][:, 0 : w0 + w1],
                        AF.Exp, scale=scale,
                    )
                    if diag:
                        # causal mask (c >= r) is all-ones beyond col 127:
                        # only the leading 128 cols of each tile need it
                        m0, m1 = min(w0, 128), min(w1, 128)
                        nc.vector.tensor_mul(
                            e_sb[:, 0:m0], e_sb[:, 0:m0],
                            ctabA_sb[:, 4096 : 4096 + m0],
                        )
                        nc.vector.tensor_mul(
                            e_sb[:, w0 : w0 + m1], e_sb[:, w0 : w0 + m1],
                            ctabA_sb[:, 4096 : 4096 + m1],
                        )
                    e_pair[par] = e_sb
                return (ph, qb, pr, e_pair, ((kt0, w0, 0), (kt1, w1, w0)), pars)

            def flush_av(u):
                ph, qb, pr, e_pair, kts, pars = u
                nkt = 4 * qb + 4
                if pr == 0:
                    avstate["pav"] = [
                        psum.tile([65, 512], F32, tag="pav", name=f"pav{i}", bufs=2)
                        for i in range(2)
                    ]
                pav = avstate["pav"]
                for kt, w, off in kts:
                    for par in pars:
                        nc.tensor.matmul(
                            pav[par][:, 512 - w : 512],
                            lhsT=v1_sb[:, kt, :],
                            rhs=e_pair[par][:, off : off + w],
                            start=(kt == 0),
                            stop=(kt == nkt - 1),
                        )
                if pr == nkt // 2 - 1:
                    emit_norm((ph, qb, pav))
                    if qb == 3:
                        # gather this head pair while later pairs compute
                        nc.gpsimd.collective_compute(
                            "AllGather", mybir.AluOpType.bypass,
                            ins=[ao_q[ph].opt()], outs=[aof_q[ph].opt()],
                            replica_groups=[[0, 1, 2, 3], [4, 5, 6, 7]],
                        )
                        if ph == 3:
                            for i in (2, 3):
                                nc.gpsimd.dma_start(
                                    aof_sb[:, i * 8192 : i * 8192 + 8192].rearrange(
                                        "p (c t) -> p c t", t=2048
                                    ),
                                    aof_q[i][:, :].rearrange(
                                        "(c p) t -> p c t", p=128
                                    ),
                                )

            # build the emission stream: attention units in order, with the
            # q projections interleaved (qp(ph,0/1) late in ph-1's stream,
            # qp(ph,2/3) after ph's qb0/qb1) so the PE never sees a long
            # exp-free projection block while ACT idles
            stream = []
            for ph in range(4):
                att = []
                for qb in range(4):
                    if ph == 0:
                        att.append(("prep", qb))
                        att.append(("qp", 0, qb))
                    att += [("att", ph, qb, pr) for pr in range(2 * qb + 2)]
                if ph == 0:
                    att.insert(len(att) - 10, ("qp", 1, 0))
                    att.insert(len(att) - 5, ("qp", 1, 1))
                else:
                    att.insert(2, ("qp", ph, 2))
                    att.insert(7, ("qp", ph, 3))
                    if ph < 3:
                        att.insert(12, ("qp", ph + 1, 0))
                        att.insert(17, ("qp", ph + 1, 1))
                stream += att
            staged = None
            for it in stream:
                if it[0] == "prep":
                    emit_prep(it[1])
                elif it[0] == "qp":
                    emit_qproj(it[1], it[2])
                else:
                    u = emit_scores_exp(it[1], it[2], it[3])
                    if staged is not None:
                        flush_av(staged)
                    staged = u
            flush_av(staged)

            # ---- output projection (512-column slice of wo) ----
            # aof chunk 4*i+c covers rank c, head pair i -> wo feature-chunk
            # 4*c+i.  Partials accumulate incrementally as each gather's
            # reload lands (pairs 0+1, then +2); only pair 3's 4 matmuls and
            # one vector add trail the final gather.
            def wos(fc, cc):
                return wo_sb[:, fc * 512 + cc * 128 : fc * 512 + cc * 128 + 128]

            def aofs(ch, t0):
                return aof_sb[:, ch * 2048 + t0 : ch * 2048 + t0 + 512]

            wo_part = acts.tile([128, 16, 512], BF)
            for cc in range(4):
                for tt in range(4):
                    ps = psum.tile([128, 512], F32, tag="mm", bufs=2)
                    n = 0
                    for i in range(2):
                        for c in range(4):
                            nc.tensor.matmul(
                                ps[:],
                                lhsT=wos(4 * c + i, cc),
                                rhs=aofs(4 * i + c, tt * 512),
                                start=(n == 0),
                                stop=(n == 7),
                            )
                            n += 1
                    nc.vector.tensor_copy(wo_part[:, 4 * cc + tt, :], ps[:])
            for cc in range(4):
                for tt in range(4):
                    ps = psum.tile([128, 512], F32, tag="mm", bufs=2)
                    for c in range(4):
                        nc.tensor.matmul(
                            ps[:],
                            lhsT=wos(4 * c + 2, cc),
                            rhs=aofs(8 + c, tt * 512),
                            start=(c == 0),
                            stop=(c == 3),
                        )
                    nc.vector.tensor_add(
                        wo_part[:, 4 * cc + tt, :], ps[:], wo_part[:, 4 * cc + tt, :]
                    )
            for tt in range(4):
                for cc in range(4):
                    ts = slice(tt * 512, tt * 512 + 512)
                    ps = psum.tile([128, 512], F32, tag="mm", bufs=2)
                    for c in range(4):
                        nc.tensor.matmul(
                            ps[:],
                            lhsT=wos(4 * c + 3, cc),
                            rhs=aofs(12 + c, tt * 512),
                            start=(c == 0),
                            stop=(c == 3),
                        )
                    o_sb = outp.tile([128, 512], BF, tag="o")
                    nc.vector.tensor_add(o_sb[:], ps[:], wo_part[:, 4 * cc + tt, :])
                    nc.sync.dma_start(outt[cc * 128 : cc * 128 + 128, ts], o_sb[:])

    return nc


def _host_tables():
    inv_freq = 1.0 / (10000.0 ** (np.arange(0, HD, 2, dtype=np.float32) / HD))
    t = np.arange(T, dtype=np.float32)
    freqs = np.einsum("i,j->ij", t, inv_freq)
    emb = np.concatenate([freqs, freqs], axis=-1)  # [T, 64]
    cosT = np.cos(emb).T.astype(np.float32)  # [64, T]
    sinT = np.sin(emb).T.astype(np.float32)

    cos2 = np.ascontiguousarray(np.vstack([cosT, cosT]))
    sin2 = np.ascontiguousarray(np.vstack([sinT, sinT]))
    coskv = np.ascontiguousarray(np.vstack([cosT, np.ones_like(cosT)]))
    sinkv = np.ascontiguousarray(np.vstack([sinT, np.zeros_like(sinT)]))

    masks = np.zeros((128, T), dtype=np.float32)
    r_idx = np.arange(128)[:, None]
    c_idx = np.arange(512)[None, :]
    for j in range(4):
        masks[:, j * 512 : j * 512 + 512] = (c_idx >= 128 * j + r_idx)

    ctabA = np.concatenate([coskv, sinkv, masks], axis=1).astype(BF16)
    ctabB = np.concatenate([cos2, sin2], axis=1).astype(BF16)

    R = np.zeros((HD, HD), dtype=np.float32)
    for d in range(32):
        R[d, d + 32] = -1.0
        R[d + 32, d] = 1.0
    r2 = np.block([[R, np.zeros_like(R)], [np.zeros_like(R), R]])
    r2t = np.ascontiguousarray(r2.T)  # lhsT: matmul computes R2 @ rhs

    ident2 = np.vstack([np.eye(HD), np.eye(HD)])  # [128, 64]
    rident = np.concatenate([r2t, ident2], axis=1).astype(BF16)  # [128, 192]

    return dict(ctabA=ctabA, ctabB=ctabB, rident=rident)


def _swz(w):
    """[2048, n] -> [128, 16*n] chunk-major swizzle (chunk fc at col fc*n)."""
    n = w.shape[1]
    return np.ascontiguousarray(
        w.reshape(16, 128, n).transpose(1, 0, 2).reshape(128, 16 * n)
    )


def prepare_in_maps(x, wq, wk, wv, wo):
    tables = _get_nc()[1]
    x = np.asarray(x, dtype=np.float32)
    wq = np.asarray(wq, dtype=np.float32)
    wk = np.asarray(wk, dtype=np.float32)
    wv = np.asarray(wv, dtype=np.float32)
    wo = np.asarray(wo, dtype=np.float32)

    # xt layout [p][tt][fc][512]
    xts = []
    for b in range(2):
        xts.append(
            np.ascontiguousarray(
                x[b]
                .reshape(4, 512, 16, 128)
                .transpose(3, 0, 2, 1)
                .reshape(128, 4 * 16 * 512)
            ).astype(BF16)
        )

    in_maps = []
    for core in range(N_CORES):
        b, g = core // 4, core % 4
        m = dict(tables)
        m["xt"] = xts[b]
        m["wq"] = _swz(wq[:, 512 * g : 512 * g + 512]).astype(BF16)
        m["wo"] = _swz(wo[:, 512 * g : 512 * g + 512]).astype(BF16)
        m["wkv"] = _swz(
            np.concatenate(
                [wk[:, 64 * g : 64 * g + 64], wv[:, 64 * g : 64 * g + 64]],
                axis=1,
            )
        ).astype(BF16)
        in_maps.append(m)
    return in_maps


def gather_output(res):
    out = np.empty((2, T, DIM), dtype=np.float32)
    for core in range(N_CORES):
        b, g = core // 4, core % 4
        out[b][:, 512 * g : 512 * g + 512] = (
            res.results[core]["outt"].astype(np.float32).T
        )
    return out


_STATE = {}


def _get_nc():
    if "nc" not in _STATE:
        _STATE["tables"] = _host_tables()
        _STATE["nc"] = _build_nc()
    return _STATE["nc"], _STATE["tables"]


def kernel(x, wq, wk, wv, wo):
    nc, _ = _get_nc()
    in_maps = prepare_in_maps(x, wq, wk, wv, wo)
    res = run_bass_kernel_spmd(
        nc, in_maps, core_ids=list(range(N_CORES)), trace=False
    )
    return gather_output(res)
